# revision 9
# baseline (speedup 1.0000x reference)
"""3-layer GCN (DropGNN inference) on 8 Trainium2 NeuronCores.

Sharding: nodes row-sharded across 8 cores (6272 rows each, padded to 50176);
edges partitioned by destination node; weights replicated. Per layer the
transformed features M = H @ W are exchanged with AllGather (Shared-output
HBM collectives) so every core can gather arbitrary source rows.

bf16 datapath: H, W, M, gathered messages and one-hot segment matrices are
bf16 (PE runs 4x faster than fp32; collective and gather bytes halve);
PSUM accumulation and the final log-softmax stay fp32.

Per core, per layer:
  dense:  49 node tiles; PE computes M tile = (H^T tile)^T @ W (H kept
          feature-major so no transposes are needed), ACT copies PSUM->SBUF
          (bf16), one DMA per half writes the shard to DRAM, AllGather.
          Sources live in two buffers (gather indices are int16-limited to
          32767 rows): half A = every core's tiles 0-24, half B = 25-48.
  edge:   two passes so the whole A-half gather sweep and PE work overlap
          the B AllGather without any cross-engine deadlock:
          pass 1 per dst tile: PSUM seeded with the self-loop contribution
          mstage_t^T @ diag(dinv^2) (self-loops never leave the core), then
          A-half 128-edge blocks; PSUM parked to SBUF as bf16.
          pass 2 per dst tile: PSUM re-seeded via identity matmul, B-half
          blocks added, then ACT relu(PSUM + b) -> next layer's H tile.
          Gathers pull 2 dst tiles per dma_gather (amortizes SWDGE fixed
          cost); DVE/ACT build per-block one-hot matrices
          onehot[e, d] = (iota[d] == dst_local[e]) * norm[e] in bf16; PE
          accumulates PSUM[f, d] += msgs[e, f]^T @ onehot[e, d].
          Next layer's dense tiles interleave into pass 2 so its AllGathers
          start as early as possible.
Final layer: PE-transpose logits to node-major, fused exp+accum log-softmax
in fp32, one DMA out. Host gathers the 8 shards and slices to 50000 rows.
"""

import os
import sys

import numpy as np

for _p in ("/opt/trn_rl_repo", "/opt/trn_rl_repo/concourse"):
    if _p not in sys.path:
        sys.path.insert(0, _p)

import ml_dtypes

BF16 = ml_dtypes.bfloat16

N = 50000
E = 625000
F_IN = 128
HID = 128
N_CLASSES = 64
NCORES = 8
TPC = 49                      # 128-node tiles per core
SH = TPC * 128                # 6272 nodes per core
NPAD = NCORES * SH            # 50176
TPA = 25                      # tiles per core in half A
SHA = TPA * 128               # 3200 rows/core in half A
SHB = SH - SHA                # 3072 rows/core in half B
NA = NCORES * SHA             # 25600 rows in buffer A
NB_ROWS = NCORES * SHB        # 24576 rows in buffer B
GRP = 2                       # dst tiles per gather instruction

LAST_RESULTS = None           # BassKernelResults of the most recent run


def _preprocess(edge_index):
    """Partition edges (self-loops excluded; handled on-chip) by destination
    tile and source half; build per-core gather indices and per-block
    (dst_local, norm) metadata plus per-node self-loop weights dinv^2.

    Global block order: for each tile group (t0, t1): A(t0), A(t1), then
    all groups done, B blocks follow in the same group order."""
    ei = np.asarray(edge_index).astype(np.int64)
    src, dst = ei[0], ei[1]

    # degrees include self-loops even though they are not in the edge lists
    deg = np.bincount(dst, minlength=NPAD).astype(np.float32)
    deg += 1.0
    deg[N:] = 0.0
    dinv = np.zeros(NPAD, np.float32)
    nz = deg > 0
    dinv[nz] = (np.float32(1.0) / np.sqrt(deg[nz])).astype(np.float32)
    norm = (dinv[src] * dinv[dst]).astype(np.float32)
    selfw = (dinv * dinv).astype(np.float32)

    order = np.argsort(dst, kind="stable")
    src_s, dst_s, nrm_s = src[order], dst[order], norm[order]
    bounds = np.searchsorted(dst_s, np.arange(0, NPAD + 1, 128))

    s_core = src_s // SH
    s_rem = src_s % SH
    in_a = s_rem < SHA
    row_a = s_core * SHA + s_rem
    row_b = s_core * SHB + (s_rem - SHA)

    ntiles = NCORES * TPC
    groups = []
    nA = np.zeros(ntiles, np.int64)
    nB = np.zeros(ntiles, np.int64)
    for g in range(ntiles):
        sl = slice(bounds[g], bounds[g + 1])
        el, en = (dst_s[sl] - g * 128), nrm_s[sl]
        mA = in_a[sl]
        a = (row_a[sl][mA], el[mA], en[mA])
        b = (row_b[sl][~mA], el[~mA], en[~mA])
        nA[g], nB[g] = len(a[0]), len(b[0])
        groups.append((a, b))

    nA2 = nA.reshape(NCORES, TPC)
    nB2 = nB.reshape(NCORES, TPC)
    kA = [int(x) for x in np.ceil(nA2.max(axis=0) / 128).astype(np.int64)]
    kB = [int(x) for x in np.ceil(nB2.max(axis=0) / 128).astype(np.int64)]
    NBLK = sum(kA) + sum(kB)

    tgroups = [list(range(t0, min(t0 + GRP, TPC)))
               for t0 in range(0, TPC, GRP)]
    acols = {}
    bcols = {}
    acc = 0
    for tg in tgroups:
        for t in tg:
            acols[t] = acc
            acc += kA[t]
    for tg in tgroups:
        for t in tg:
            bcols[t] = acc
            acc += kB[t]
    assert acc == NBLK

    idx_all, mdst_all, mnrm_all, selfw_all = [], [], [], []
    for c in range(NCORES):
        idx16 = np.zeros((16, NBLK * 8), np.int16)
        mdst = np.full((128, NBLK), -1.0, np.float32)
        mnrm = np.zeros((128, NBLK), np.float32)
        for t in range(TPC):
            a, b = groups[c * TPC + t]
            for (es, el, en), k, bcol in ((a, kA[t], acols[t]),
                                          (b, kB[t], bcols[t])):
                if k == 0:
                    continue
                n = len(es)
                if n:
                    s = np.arange(n)
                    mdst[s % 128, bcol + s // 128] = el.astype(np.float32)
                    mnrm[s % 128, bcol + s // 128] = en
                j = np.arange(k * 128)
                vals = np.zeros(k * 128, np.int16)
                vals[:n] = es.astype(np.int16)
                idx16[j % 16, bcol * 8 + j // 16] = vals
        idx_all.append(np.tile(idx16, (8, 1)))
        mdst_all.append(mdst)
        mnrm_all.append(mnrm)
        sw = selfw[c * SH:(c + 1) * SH].reshape(TPC, 128).T
        selfw_all.append(np.ascontiguousarray(sw))

    return (kA, kB, NBLK, tgroups, acols, bcols, idx_all, mdst_all,
            mnrm_all, selfw_all)


def _build(kA, kB, NBLK, tgroups, acols, bcols):
    import concourse.bacc as bacc
    import concourse.mybir as mybir
    import concourse.tile as tile
    from concourse import masks

    f32 = mybir.dt.float32
    b16 = mybir.dt.bfloat16
    i16 = mybir.dt.int16
    Alu = mybir.AluOpType
    Act = mybir.ActivationFunctionType

    nc = bacc.Bacc(
        None, target_bir_lowering=False, num_devices=NCORES,
        num_swdge_queues=4,
    )

    xT_d = nc.dram_tensor("xT", [128, SH], b16, kind="ExternalInput")
    w1_d = nc.dram_tensor("w1", [128, HID], b16, kind="ExternalInput")
    w2_d = nc.dram_tensor("w2", [128, HID], b16, kind="ExternalInput")
    wf_d = nc.dram_tensor("wf", [128, HID], b16, kind="ExternalInput")
    bias_d = nc.dram_tensor("bias", [128, 3], f32, kind="ExternalInput")
    iota_d = nc.dram_tensor("iota", [128, 128], b16, kind="ExternalInput")
    mdst_d = nc.dram_tensor("mdst", [128, NBLK], f32, kind="ExternalInput")
    mnrm_d = nc.dram_tensor("mnrm", [128, NBLK], f32, kind="ExternalInput")
    selfw_d = nc.dram_tensor("selfw", [128, TPC], f32, kind="ExternalInput")
    iotac_d = nc.dram_tensor("iotac", [128, 1], f32, kind="ExternalInput")
    idx_d = nc.dram_tensor("idx", [128, NBLK * 8], i16, kind="ExternalInput")
    out_d = nc.dram_tensor("logits", [SH, N_CLASSES], f32,
                           kind="ExternalOutput")

    # two buffer pairs; L3 reuses L1's (safe: any core's L3 AllGather
    # transitively waits on every core's L1 gathers through the L2 AllGather)
    _space = "Local" if os.environ.get("KERNEL_AG_LOCAL") else "Shared"
    _mga = [nc.dram_tensor(f"mga{i}", [NA, HID], b16, kind="Internal",
                           addr_space=_space) for i in range(2)]
    _mgb = [nc.dram_tensor(f"mgb{i}", [NB_ROWS, HID], b16, kind="Internal",
                           addr_space=_space) for i in range(2)]
    mga_d = [_mga[0], _mga[1], _mga[0]]
    mgb_d = [_mgb[0], _mgb[1], _mgb[0]]

    rg = [list(range(NCORES))]

    with tile.TileContext(nc, num_cores=NCORES) as tc:
        with (
            tc.tile_pool(name="const", bufs=1) as cp,
            tc.tile_pool(name="hbuf", bufs=1) as hp,
            tc.tile_pool(name="stage", bufs=1) as sp,
            tc.tile_pool(name="msga", bufs=4) as mpa,
            tc.tile_pool(name="msgb", bufs=4) as mpb,
            tc.tile_pool(name="oh", bufs=8) as op_,
            tc.tile_pool(name="lt", bufs=4) as lp,
            tc.tile_pool(name="small", bufs=4) as zp,
            tc.tile_pool(name="pm", bufs=2, space="PSUM") as pmp,
            tc.tile_pool(name="ph", bufs=4, space="PSUM") as php,
            tc.tile_pool(name="pt", bufs=2, space="PSUM") as ptp,
            tc.tile_pool(name="dram", bufs=1, space="DRAM") as dp,
        ):
            w1 = cp.tile([128, HID], b16)
            w2 = cp.tile([128, HID], b16)
            wf = cp.tile([128, HID], b16)
            bias = cp.tile([128, 3], f32)
            iota = cp.tile([128, 128], b16)
            mdst = cp.tile([128, NBLK], f32)
            mnrm = cp.tile([128, NBLK], f32)
            selfw = cp.tile([128, TPC], f32)
            iotac = cp.tile([128, 1], f32)
            idxs = cp.tile([128, NBLK * 8], i16)
            ident = cp.tile([128, 128], f32)
            identb = cp.tile([128, 128], b16)
            mdstn = cp.tile([128, NBLK], f32)   # -mdst (ACT one-hot path)
            mnrmn = cp.tile([128, NBLK], f32)   # -mnrm

            h0 = hp.tile([128, SH], b16, tag="h0")
            h1 = hp.tile([128, SH], b16, tag="h1")
            mstage = sp.tile([128, TPC * HID], b16, tag="mst")
            paccum = sp.tile([128, TPC * 128], b16, tag="pacc")
            logits = sp.tile([128, TPC * N_CLASSES], f32, tag="lg")
            xs = sp.tile([128, TPC * N_CLASSES], f32, tag="xs")
            sums = sp.tile([128, TPC], f32, tag="sm")
            lsum = sp.tile([128, TPC], f32, tag="ls")

            nc.sync.dma_start(w1[:], w1_d[:])
            nc.sync.dma_start(w2[:], w2_d[:])
            nc.sync.dma_start(wf[:], wf_d[:])
            nc.sync.dma_start(bias[:], bias_d[:])
            nc.sync.dma_start(iota[:], iota_d[:])
            nc.sync.dma_start(mdst[:], mdst_d[:])
            nc.sync.dma_start(mnrm[:], mnrm_d[:])
            nc.sync.dma_start(selfw[:], selfw_d[:])
            nc.sync.dma_start(iotac[:], iotac_d[:])
            nc.sync.dma_start(idxs[:], idx_d[:])
            nc.sync.dma_start(h0[:], xT_d[:])
            masks.make_identity(nc, ident[:])
            nc.vector.tensor_copy(identb[:], ident[:])
            nc.vector.tensor_scalar_mul(mdstn[:], mdst[:], -1.0)
            nc.vector.tensor_scalar_mul(mnrmn[:], mnrm[:], -1.0)

            weights = (w1, w2, wf)

            def build_oh(oh, g):
                # one-hot block g: oh[e, d] = (iota[d] == dst[e]) * norm[e]
                if g % 6 == 0:
                    nc.scalar.activation(
                        oh[:], iota[:], Act.Abs, bias=mdstn[:, g:g + 1],
                    )
                    nc.scalar.activation(
                        oh[:], oh[:], Act.Relu,
                        bias=mnrm[:, g:g + 1], scale=mnrmn[:, g:g + 1],
                    )
                else:
                    nc.vector.tensor_scalar(
                        oh[:], iota[:], mdst[:, g:g + 1], mnrm[:, g:g + 1],
                        Alu.is_equal, Alu.mult,
                    )

            def dense_tile(L, t, hsrc):
                pm = pmp.tile([128, HID], f32, tag="pm")
                nc.tensor.matmul(
                    pm[:], hsrc[:, t * 128:(t + 1) * 128], weights[L][:],
                    start=True, stop=True,
                )
                nc.scalar.activation(
                    mstage[:, t * HID:(t + 1) * HID], pm[:], Act.Copy
                )

            def ship_half(L, half):
                if half == 0:
                    msh = dp.tile([SHA, HID], b16, tag=f"msha{L}")
                    st = mstage[:, :TPA * HID]
                    mg = mga_d[L]
                else:
                    msh = dp.tile([SHB, HID], b16, tag=f"mshb{L}")
                    st = mstage[:, TPA * HID:]
                    mg = mgb_d[L]
                nc.sync.dma_start(
                    msh[:].rearrange("(t p) f -> p t f", p=128),
                    st.rearrange("p (t f) -> p t f", f=HID),
                )
                nc.gpsimd.collective_compute(
                    "AllGather", mybir.AluOpType.bypass,
                    replica_groups=rg,
                    ins=[msh[:].opt()],
                    outs=[mg[:].opt()],
                )

            def edge_pass1(L):
                # A-half gathers + PSUM(self + A blocks) -> paccum (bf16)
                p1 = os.environ.get("KERNEL_P1", "full")
                for tg in tgroups:
                    ka = sum(kA[t] for t in tg)
                    ma = None
                    if ka:
                        ma = mpa.tile([128, ka, HID], b16, tag="ma")
                        c0 = acols[tg[0]]
                        nc.gpsimd.dma_gather(
                            ma[:], mga_d[L][:, :],
                            idxs[:, c0 * 8:(c0 + ka) * 8],
                            ka * 128, ka * 128, HID,
                            queue_num=(2 * tg[0]) % 4,
                        )
                    if p1 == "gather":
                        if ma is not None:
                            jt = zp.tile([128, HID], b16, tag="jt")
                            nc.vector.tensor_copy(jt[:], ma[:, 0, :])
                        continue
                    aoff = 0
                    for t in tg:
                        soh = op_.tile([128, 128], b16, tag="soh")
                        nc.vector.tensor_scalar(
                            soh[:], iota[:], iotac[:], selfw[:, t:t + 1],
                            Alu.is_equal, Alu.mult,
                        )
                        if p1 == "oh":
                            for b in range(kA[t]):
                                oh = op_.tile([128, 128], b16, tag="oh")
                                build_oh(oh, acols[t] + b)
                            continue
                        ph = php.tile([HID, 128], f32, tag="ph")
                        nc.tensor.matmul(
                            ph[:], mstage[:, t * HID:(t + 1) * HID], soh[:],
                            start=True,
                            stop=(kA[t] == 0 or p1 == "self"),
                        )
                        if p1 != "self":
                            for b in range(kA[t]):
                                g = acols[t] + b
                                oh = op_.tile([128, 128], b16, tag="oh")
                                build_oh(oh, g)
                                nc.tensor.matmul(
                                    ph[:], ma[:, aoff + b, :], oh[:],
                                    start=False, stop=(b == kA[t] - 1),
                                )
                        aoff += kA[t]
                        nc.scalar.activation(
                            paccum[:, t * 128:(t + 1) * 128], ph[:], Act.Copy
                        )

            def edge_pass2(L, hdst):
                # B-half gathers + PSUM(seed + B blocks) -> epilogue; next
                # layer's dense tiles and ships interleave here.
                for tg in tgroups:
                    kb = sum(kB[t] for t in tg)
                    mb = None
                    if kb:
                        mb = mpb.tile([128, kb, HID], b16, tag="mb")
                        c0 = bcols[tg[0]]
                        nc.gpsimd.dma_gather(
                            mb[:], mgb_d[L][:, :],
                            idxs[:, c0 * 8:(c0 + kb) * 8],
                            kb * 128, kb * 128, HID,
                            queue_num=(2 * tg[0] + 1) % 4,
                        )
                    boff = 0
                    for t in tg:
                        ph = php.tile([HID, 128], f32, tag="ph")
                        nc.tensor.matmul(
                            ph[:], identb[:],
                            paccum[:, t * 128:(t + 1) * 128],
                            start=True, stop=(kB[t] == 0),
                        )
                        for b in range(kB[t]):
                            g = bcols[t] + b
                            oh = op_.tile([128, 128], b16, tag="oh")
                            build_oh(oh, g)
                            nc.tensor.matmul(
                                ph[:], mb[:, boff + b, :], oh[:],
                                start=False, stop=(b == kB[t] - 1),
                            )
                        boff += kB[t]
                        if L < 2:
                            nc.scalar.activation(
                                hdst[:, t * 128:(t + 1) * 128], ph[:],
                                Act.Relu, bias=bias[:, L:L + 1],
                            )
                            dense_tile(L + 1, t, hdst)
                            if t == TPA - 1:
                                ship_half(L + 1, 0)
                            elif t == TPC - 1:
                                ship_half(L + 1, 1)
                        else:
                            lt = lp.tile([N_CLASSES, 128], f32, tag="lt")
                            nc.scalar.activation(
                                lt[:], ph[:N_CLASSES, :], Act.Identity,
                                bias=bias[:N_CLASSES, 2:3],
                            )
                            pt = ptp.tile([128, N_CLASSES], f32, tag="pt")
                            nc.tensor.transpose(
                                pt[:], lt[:], ident[:N_CLASSES, :N_CLASSES]
                            )
                            nc.vector.tensor_copy(
                                logits[:, t * N_CLASSES:(t + 1) * N_CLASSES],
                                pt[:],
                            )

            # ---- layer pipeline ----
            nl = int(os.environ.get("KERNEL_NLAYERS", "3"))
            stage = os.environ.get("KERNEL_STAGE", "full")
            do_softmax = os.environ.get("KERNEL_SOFTMAX", "1") != "0"
            hcur, hnxt = h0, h1
            for t in range(TPA):
                dense_tile(0, t, hcur)
            if stage != "dense":
                ship_half(0, 0)
            for t in range(TPA, TPC):
                dense_tile(0, t, hcur)
            if stage != "dense":
                ship_half(0, 1)

            if nl < 3 or stage != "full":
                nc.vector.memset(logits[:], 0.0)
            if stage in ("full", "pass1", "pass2"):
                for L in range(3 - nl, 3):
                    edge_pass1(L)
                    if stage == "pass1":
                        break
                    edge_pass2(L, hnxt)
                    hcur, hnxt = hnxt, hcur

            # ---- log-softmax over the 64 classes (free dim, node-major) ----
            lg3 = logits[:].rearrange("p (t c) -> p t c", c=N_CLASSES)
            xs3 = xs[:].rearrange("p (t c) -> p t c", c=N_CLASSES)
            on3 = lg3
            if do_softmax:
                for t in range(TPC):
                    mx = zp.tile([128, 1], f32, tag="mx")
                    nc.vector.tensor_reduce(
                        mx[:], lg3[:, t, :], mybir.AxisListType.X, Alu.max
                    )
                    nc.vector.tensor_scalar(
                        xs3[:, t, :], lg3[:, t, :], mx[:], None, Alu.subtract
                    )
                    junk = zp.tile([128, N_CLASSES], f32, tag="jk")
                    nc.scalar.activation(
                        junk[:], xs3[:, t, :], Act.Exp,
                        accum_out=sums[:, t:t + 1],
                    )
                nc.scalar.activation(lsum[:], sums[:], Act.Ln)
                for t in range(TPC):
                    nc.vector.tensor_scalar(
                        on3[:, t, :], xs3[:, t, :], lsum[:, t:t + 1], None,
                        Alu.subtract,
                    )
            nc.sync.dma_start(
                out_d[:].rearrange("(t p) c -> p t c", p=128),
                on3,
            )

    nc.compile()
    return nc


def kernel(x, edge_index, W1, b1, W2, b2, Wf, bf):
    global LAST_RESULTS
    from concourse.bass_utils import run_bass_kernel_spmd

    x = np.asarray(x, dtype=np.float32)
    W1 = np.asarray(W1, dtype=np.float32)
    b1 = np.asarray(b1, dtype=np.float32)
    W2 = np.asarray(W2, dtype=np.float32)
    b2 = np.asarray(b2, dtype=np.float32)
    Wf = np.asarray(Wf, dtype=np.float32)
    bf = np.asarray(bf, dtype=np.float32)

    (kA, kB, NBLK, tgroups, acols, bcols, idx_all, mdst_all, mnrm_all,
     selfw_all) = _preprocess(edge_index)
    nc = _build(kA, kB, NBLK, tgroups, acols, bcols)

    xpad = np.zeros((NPAD, F_IN), np.float32)
    xpad[:N] = x
    bias = np.zeros((128, 3), np.float32)
    bias[:, 0] = b1
    bias[:, 1] = b2
    bias[:N_CLASSES, 2] = bf
    iota = np.tile(np.arange(128, dtype=np.float32), (128, 1))
    iotac = np.arange(128, dtype=np.float32).reshape(128, 1)

    in_maps = []
    for c in range(NCORES):
        in_maps.append({
            "xT": np.ascontiguousarray(
                xpad[c * SH:(c + 1) * SH].T).astype(BF16),
            "w1": W1.astype(BF16), "w2": W2.astype(BF16),
            "wf": np.pad(Wf, ((0, 0), (0, HID - N_CLASSES))).astype(BF16),
            "bias": bias, "iota": iota.astype(BF16), "iotac": iotac,
            "selfw": selfw_all[c],
            "mdst": mdst_all[c], "mnrm": mnrm_all[c], "idx": idx_all[c],
        })

    res = run_bass_kernel_spmd(
        nc, in_maps, core_ids=list(range(NCORES)),
        trace=bool(os.environ.get("BASS_TRACE")),
    )
    LAST_RESULTS = res
    if res.exec_time_ns is not None:
        print(f"HW exec time: {res.exec_time_ns} ns")

    out = np.concatenate([r["logits"] for r in res.results], axis=0)
    return out[:N].astype(np.float32)
